# revision 1
# baseline (speedup 1.0000x reference)
"""Expert-parallel MoE (top-2 of 16 experts) for Trainium2, 8 NeuronCores.

Sharding strategy (per spec sharding_hint): expert-parallel. The 16 experts'
gate/up/down weights are sharded 2-per-core across the 8 cores. The router
(a [T,16] matmul + top-2, ~0.01% of total FLOPs) runs on the host at input-
shard time; the "all-to-all token dispatch" is realized as the host-side
gather that builds each core's token batch, and the top-2 weighted combine
is the host-side scatter-add at unshard time.

On-device per core (all heavy FLOPs + memory traffic):
  for each of its 2 experts, with X_e^T [H, C] (tokens on the PE free dim):
    G1 = gate_w[:, :2048]^T-tiles @ X^T      (PSUM f32)
    G2 = silu(gate_w[:, 2048:] @ X^T)        (ScalarE silu from PSUM)
    HH = G2 * G1 * (up_w @ X^T)              (VectorE, bf16)
    Y^T = down_w-tiles @ HH                  (PSUM f32 -> SBUF -> HBM)

All matmuls in bf16 (weights cast at shard time), f32 accumulation.
Activations are laid out transposed ([H, C], tokens on the moving/free dim)
so every weight matrix is used in its natural [K, M] layout with zero
on-device transposes.
"""

import os

import numpy as np
import ml_dtypes

import concourse.tile as tile
import concourse.mybir as mybir
from concourse import bacc
from concourse import bass_utils

N_CORES = 8
E = 16
H = 1024
I_G = 4096  # gate projection width
I_H = 2048  # up/down inner width
KB_H = H // 128  # 8 k-tiles for H-contraction
KB_I = I_H // 128  # 16 k-tiles for I_H-contraction

# 16-bit matmul dtype: fp16 and bf16 run at the same PE rate (1 cyc/row);
# fp16's 10 mantissa bits give ~4x lower rounding error for this data
# (all values well inside fp16 range).
BF16 = mybir.dt.float16
F32 = mybir.dt.float32
NP_BF16 = np.float16


def _ceil_mult(n: int, m: int) -> int:
    return ((n + m - 1) // m) * m


def _split_c(C: int):
    """Split capacity C (multiple of 128) into PE free-dim tiles.

    Tiles are kept in {512, 384, 256} where possible: <=512 fits one PSUM
    bank in f32; >=256 keeps the per-tile LDWEIGHTS (~107ns) hidden under
    the matmul stream. Returns list of (offset, width)."""
    assert C % 128 == 0 and C > 0
    if C <= 512:
        widths = [C]
    else:
        q, r = divmod(C, 512)
        if r == 0:
            widths = [512] * q
        elif r == 256:
            widths = [512] * q + [256]
        elif r == 384:
            widths = [512] * q + [384]
        else:  # r == 128
            widths = [512] * (q - 1) + [384, 256]
    out = []
    off = 0
    for w in widths:
        out.append((off, w))
        off += w
    assert off == C
    return out


def _chunk2(seq):
    return [seq[i : i + 2] for i in range(0, len(seq), 2)]


_OPT = dict(psum_merged=True, dma_split=False, y_chunk=False, head_split=8, wp_bufs=6, tp_bufs=4, yp_bufs=3)


def _load_slab(nc, pool, shape, src, tag, name, parts=2):
    """Allocate a [128, kb, n] slab and load it with `parts` DMAs split
    along the k dimension, so the first k-tiles land early and the PE can
    start before the whole slab arrives."""
    t = pool.tile(shape, BF16, tag=tag, name=name)
    kb = shape[1]
    step = max(1, kb // parts)
    for a in range(0, kb, step):
        b = min(kb, a + step)
        nc.sync.dma_start(out=t[:, a:b, :], in_=src[:, a:b, :])
    return t


def _ptag(name):
    return "ps" if _OPT["psum_merged"] else name


def _expert_ffn(nc, wp, xp, hp, yp, tp, pp, x, g, u, d, y, C, first=False):
    """Emit one expert's FFN: y[H, C] = down( silu(g2)*g1*up ) for x[H, C]."""
    ct = _split_c(C)

    xr = x.rearrange("(kb p) c -> p kb c", p=128)  # [128, 8, C]
    gr = g.rearrange("(kb p) i -> p kb i", p=128)  # [128, 8, 4096]
    ur = u.rearrange("(kb p) i -> p kb i", p=128)  # [128, 8, 2048]
    dr = d.rearrange("(kb p) h -> p kb h", p=128)  # [128, 16, 1024]
    yr = y.rearrange("(hb p) c -> p hb c", p=128)  # [128, 8, C]

    hs_parts = _OPT["head_split"] if first else 1
    xs = xp.tile([128, KB_H, C], BF16, tag="xt", name="xs")
    sg2_0 = None
    if hs_parts > 1:
        # First expert: the PE's very first LDW/MM needs sg2[k=0] and
        # xs[k=0]. Interleave their k-slice DMAs so the earliest-needed
        # pieces land on distinct queues in the first round-robin wave.
        sg2_0 = wp.tile([128, KB_H, 1024], BF16, tag="w", name="sg2")
        for k in range(KB_H):
            nc.sync.dma_start(
                out=sg2_0[:, k : k + 1, :],
                in_=gr[:, k : k + 1, 2048 : 2048 + 1024],
            )
            nc.sync.dma_start(out=xs[:, k : k + 1, :], in_=xr[:, k : k + 1, :])
    else:
        nc.sync.dma_start(out=xs, in_=xr)

    hh = hp.tile([128, KB_I, C], BF16, tag="hh", name="hh")

    # ---- gate + up fused phase ----
    for half in range(2):  # hh i-tiles 0-7 / 8-15
        lo = half * 1024
        p = hs_parts if half == 0 else 1
        if half == 0 and sg2_0 is not None:
            sg2 = sg2_0
        else:
            sg2 = _load_slab(
                nc, wp, [128, KB_H, 1024],
                gr[:, :, 2048 + lo : 2048 + lo + 1024], "w", "sg2", parts=p,
            )
        sg1 = _load_slab(
            nc, wp, [128, KB_H, 1024], gr[:, :, lo : lo + 1024], "w", "sg1",
            parts=p,
        )
        su = _load_slab(
            nc, wp, [128, KB_H, 1024], ur[:, :, lo : lo + 1024], "w", "su",
            parts=p,
        )

        for il in range(8):
            i = half * 8 + il
            ms = slice(il * 128, (il + 1) * 128)
            for cc in _chunk2(ct):
                # --- g2 stream (silu half) ---
                pg2 = [
                    pp.tile([128, w], F32, tag=_ptag("pg2"), name="pg2") for (_, w) in cc
                ]
                for k in range(KB_H):
                    for j, (off, w) in enumerate(cc):
                        nc.tensor.matmul(
                            pg2[j],
                            sg2[:, k, ms],
                            xs[:, k, off : off + w],
                            start=(k == 0),
                            stop=(k == KB_H - 1),
                        )
                sil = []
                for j, (off, w) in enumerate(cc):
                    t = tp.tile([128, 512], BF16, tag="t", name="t")
                    nc.scalar.activation(
                        out=t[:, :w],
                        in_=pg2[j],
                        func=mybir.ActivationFunctionType.Silu,
                    )
                    sil.append(t)
                # --- g1 stream ---
                pg1 = [
                    pp.tile([128, w], F32, tag=_ptag("pg1"), name="pg1") for (_, w) in cc
                ]
                for k in range(KB_H):
                    for j, (off, w) in enumerate(cc):
                        nc.tensor.matmul(
                            pg1[j],
                            sg1[:, k, ms],
                            xs[:, k, off : off + w],
                            start=(k == 0),
                            stop=(k == KB_H - 1),
                        )
                g12 = []
                for j, (off, w) in enumerate(cc):
                    t2 = tp.tile([128, 512], BF16, tag="g12", name="t2")
                    nc.vector.tensor_mul(t2[:, :w], sil[j][:, :w], pg1[j])
                    g12.append(t2)
                # --- up stream ---
                pu = [
                    pp.tile([128, w], F32, tag=_ptag("pu"), name="pu") for (_, w) in cc
                ]
                for k in range(KB_H):
                    for j, (off, w) in enumerate(cc):
                        nc.tensor.matmul(
                            pu[j],
                            su[:, k, ms],
                            xs[:, k, off : off + w],
                            start=(k == 0),
                            stop=(k == KB_H - 1),
                        )
                for j, (off, w) in enumerate(cc):
                    nc.vector.tensor_mul(
                        hh[:, i, off : off + w], g12[j][:, :w], pu[j]
                    )

    # ---- down phase ----
    dlo = _load_slab(nc, wp, [128, KB_H, 1024], dr[:, 0:8, :], "w", "dlo")
    dhi = _load_slab(nc, wp, [128, KB_H, 1024], dr[:, 8:16, :], "w", "dhi")

    for h in range(8):
        ms = slice(h * 128, (h + 1) * 128)
        yl = yp.tile([128, C], F32, tag="y", name="yl")
        for cc in _chunk2(ct):
            pd = [
                pp.tile([128, w], F32, tag=_ptag("pd"), name="pd") for (_, w) in cc
            ]
            for k in range(KB_I):
                sl = dlo if k < 8 else dhi
                for j, (off, w) in enumerate(cc):
                    nc.tensor.matmul(
                        pd[j],
                        sl[:, k % 8, ms],
                        hh[:, k, off : off + w],
                        start=(k == 0),
                        stop=(k == KB_I - 1),
                    )
            for j, (off, w) in enumerate(cc):
                nc.vector.tensor_copy(yl[:, off : off + w], pd[j])
                if _OPT["y_chunk"]:
                    nc.sync.dma_start(
                        out=yr[:, h, off : off + w], in_=yl[:, off : off + w]
                    )
        if not _OPT["y_chunk"]:
            nc.sync.dma_start(out=yr[:, h, :], in_=yl)


def _build_nc(CA: int, CB: int, reps: int = 1):
    """Build + compile the 2-expert-slot SPMD program (same on all cores)."""
    nc = bacc.Bacc(
        "TRN2", target_bir_lowering=False, debug=False, num_devices=N_CORES
    )
    dram = {}
    for s, C in (("a", CA), ("b", CB)):
        dram[f"x{s}"] = nc.dram_tensor(
            f"x{s}", [H, C], BF16, kind="ExternalInput"
        ).ap()
        dram[f"g{s}"] = nc.dram_tensor(
            f"g{s}", [H, I_G], BF16, kind="ExternalInput"
        ).ap()
        dram[f"u{s}"] = nc.dram_tensor(
            f"u{s}", [H, I_H], BF16, kind="ExternalInput"
        ).ap()
        dram[f"d{s}"] = nc.dram_tensor(
            f"d{s}", [I_H, H], BF16, kind="ExternalInput"
        ).ap()
        dram[f"y{s}"] = nc.dram_tensor(
            f"y{s}", [H, C], F32, kind="ExternalOutput"
        ).ap()

    with tile.TileContext(nc) as tc:
        with (
            tc.tile_pool(name="wp", bufs=_OPT["wp_bufs"]) as wp,  # 16KB/part weight slabs
            tc.tile_pool(name="xp", bufs=2) as xp,
            tc.tile_pool(name="hp", bufs=1) as hp,
            tc.tile_pool(name="yp", bufs=_OPT["yp_bufs"]) as yp,
            tc.tile_pool(name="tp", bufs=_OPT["tp_bufs"]) as tp,
            tc.tile_pool(name="pp", bufs=(8 if _OPT["psum_merged"] else 2), space="PSUM") as pp,
        ):

            def body():
                for si, (s, C) in enumerate((("a", CA), ("b", CB))):
                    _expert_ffn(
                        nc,
                        wp,
                        xp,
                        hp,
                        yp,
                        tp,
                        pp,
                        dram[f"x{s}"],
                        dram[f"g{s}"],
                        dram[f"u{s}"],
                        dram[f"d{s}"],
                        dram[f"y{s}"],
                        C,
                        first=(si == 0),
                    )

            if reps == 1:
                body()
            else:
                ET = mybir.EngineType
                with tc.For_i(
                    0,
                    reps,
                    1,
                    hint_engines=(ET.PE, ET.DVE, ET.Activation, ET.SP, ET.Pool),
                ):
                    body()

    nc.compile()
    return nc


_NC_CACHE: dict = {}


def _get_nc(CA: int, CB: int):
    key = (CA, CB)
    if key not in _NC_CACHE:
        _NC_CACHE[key] = _build_nc(CA, CB)
    return _NC_CACHE[key]


def _route_and_shard(hs, rw, gw, uw, dw):
    """Host-side router + expert-parallel sharding of the full inputs."""
    B, S, _ = hs.shape
    T = B * S
    x = np.ascontiguousarray(hs.reshape(T, H)).astype(np.float32, copy=False)

    logits = x @ rw.astype(np.float32)  # [T, E]
    order = np.argsort(-logits, axis=1, kind="stable")[:, :2]
    l12 = np.take_along_axis(logits, order, axis=1).astype(np.float64)
    w1 = 1.0 / (1.0 + np.exp(l12[:, 1] - l12[:, 0]))  # renormalized top-2
    w2 = 1.0 - w1
    i1, i2 = order[:, 0], order[:, 1]

    idx, wts = [], []
    for e in range(E):
        m1 = i1 == e
        ide = np.nonzero(m1 | (i2 == e))[0]
        we = np.where(m1[ide], w1[ide], w2[ide]).astype(np.float32)
        idx.append(ide)
        wts.append(we)
    counts = np.array([len(v) for v in idx])

    # pair largest with smallest expert per core for load balance
    desc = np.argsort(-counts, kind="stable")
    slotA = [int(desc[c]) for c in range(N_CORES)]
    slotB = [int(desc[2 * N_CORES - 1 - c]) for c in range(N_CORES)]
    CA = max(256, _ceil_mult(int(counts[slotA].max()), 128))
    CB = max(256, _ceil_mult(int(counts[slotB].max()), 128))

    xT = np.ascontiguousarray(x.T).astype(NP_BF16)  # [H, T]

    in_maps = []
    for c in range(N_CORES):
        m = {}
        for s, e, C in (("a", slotA[c], CA), ("b", slotB[c], CB)):
            xe = np.zeros((H, C), NP_BF16)
            n = counts[e]
            xe[:, :n] = xT[:, idx[e]]
            m[f"x{s}"] = xe
            m[f"g{s}"] = np.ascontiguousarray(gw[e]).astype(NP_BF16)
            m[f"u{s}"] = np.ascontiguousarray(uw[e]).astype(NP_BF16)
            m[f"d{s}"] = np.ascontiguousarray(dw[e]).astype(NP_BF16)
        in_maps.append(m)

    meta = dict(
        B=B, S=S, T=T, idx=idx, wts=wts, counts=counts,
        slotA=slotA, slotB=slotB, CA=CA, CB=CB,
    )
    return in_maps, meta


def _combine(results, meta):
    """Host-side top-2 weighted combine (unshard)."""
    T = meta["T"]
    out = np.zeros((T, H), np.float32)
    for c in range(N_CORES):
        for s, e in (("a", meta["slotA"][c]), ("b", meta["slotB"][c])):
            n = int(meta["counts"][e])
            if n == 0:
                continue
            y = results[c][f"y{s}"][:, :n]  # [H, n] f32
            out[meta["idx"][e]] += meta["wts"][e][:, None] * y.T
    return out.reshape(meta["B"], meta["S"], H)


def _run_spmd(nc, in_maps):
    try:
        return bass_utils.run_bass_kernel_spmd(
            nc, in_maps, core_ids=list(range(N_CORES))
        )
    except ModuleNotFoundError:
        # axon NTFF profiling hook unavailable in this container; retry
        # with tracing force-disabled.
        os.environ["BASS_NEVER_TRACE"] = "1"
        try:
            return bass_utils.run_bass_kernel_spmd(
                nc, in_maps, core_ids=list(range(N_CORES))
            )
        finally:
            os.environ.pop("BASS_NEVER_TRACE", None)


def kernel(hidden_states, router_w, gate_w, up_w, down_w):
    hs = np.asarray(hidden_states)
    rw = np.asarray(router_w)
    gw = np.asarray(gate_w)
    uw = np.asarray(up_w)
    dw = np.asarray(down_w)

    in_maps, meta = _route_and_shard(hs, rw, gw, uw, dw)
    nc = _get_nc(meta["CA"], meta["CB"])
    res = _run_spmd(nc, in_maps)
    return _combine(res.results, meta)



# revision 3
# speedup vs baseline: 1.0847x; 1.0847x over previous
"""Expert-parallel MoE (top-2 of 16 experts) for Trainium2, 8 NeuronCores.

Sharding strategy (per spec sharding_hint): expert-parallel. The 16 experts'
gate/up/down weights are sharded 2-per-core across the 8 cores. The router
(a [T,16] matmul + top-2, ~0.01% of total FLOPs) runs on the host at input-
shard time; the "all-to-all token dispatch" is realized as the host-side
gather that builds each core's token batch, and the top-2 weighted combine
is the host-side scatter-add at unshard time.

On-device per core (all heavy FLOPs + memory traffic):
  for each of its 2 experts, with X_e^T [H, C] (tokens on the PE free dim):
    G1 = gate_w[:, :2048]^T-tiles @ X^T      (PSUM f32)
    G2 = silu(gate_w[:, 2048:] @ X^T)        (ScalarE silu from PSUM)
    HH = G2 * G1 * (up_w @ X^T)              (VectorE, bf16)
    Y^T = down_w-tiles @ HH                  (PSUM f32 -> SBUF -> HBM)

All matmuls in bf16 (weights cast at shard time), f32 accumulation.
Activations are laid out transposed ([H, C], tokens on the moving/free dim)
so every weight matrix is used in its natural [K, M] layout with zero
on-device transposes.
"""

import os

import numpy as np
import ml_dtypes

import concourse.tile as tile
import concourse.mybir as mybir
from concourse import bacc
from concourse import bass_utils

N_CORES = 8
E = 16
H = 1024
I_G = 4096  # gate projection width
I_H = 2048  # up/down inner width
KB_H = H // 128  # 8 k-tiles for H-contraction
KB_I = I_H // 128  # 16 k-tiles for I_H-contraction

# 16-bit matmul dtype: fp16 and bf16 run at the same PE rate (1 cyc/row);
# fp16's 10 mantissa bits give ~4x lower rounding error for this data
# (all values well inside fp16 range).
BF16 = mybir.dt.float16
F32 = mybir.dt.float32
NP_BF16 = np.float16


def _ceil_mult(n: int, m: int) -> int:
    return ((n + m - 1) // m) * m


def _split_c(C: int):
    """Split capacity C (multiple of 4) into PE free-dim tiles.

    Near-equal chunks, each <=512 (one PSUM bank in f32) and >=256 where
    possible (keeps the per-tile LDWEIGHTS (~107ns) hidden under the
    matmul stream). Returns list of (offset, width)."""
    assert C % 4 == 0 and C > 0
    n = (C + 511) // 512
    base, r = divmod(C, n)
    # distribute remainder over the first r chunks, keep widths mult-of-4
    widths = []
    acc = 0
    for i in range(n):
        w = C * (i + 1) // n - acc
        w = (w + 3) // 4 * 4
        w = min(w, C - acc)
        widths.append(w)
        acc += w
    assert acc == C and all(w <= 512 for w in widths)
    out = []
    off = 0
    for w in widths:
        out.append((off, w))
        off += w
    return out


def _chunk2(seq):
    return [seq[i : i + 2] for i in range(0, len(seq), 2)]


_OPT = dict(psum_merged=True, dma_split=False, y_chunk=False, head_split=8, wp_bufs=6, tp_bufs=4, yp_bufs=3)


def _load_slab(nc, pool, shape, src, tag, name, parts=2):
    """Allocate a [128, kb, n] slab and load it with `parts` DMAs split
    along the k dimension, so the first k-tiles land early and the PE can
    start before the whole slab arrives."""
    t = pool.tile(shape, BF16, tag=tag, name=name)
    kb = shape[1]
    step = max(1, kb // parts)
    for a in range(0, kb, step):
        b = min(kb, a + step)
        nc.sync.dma_start(out=t[:, a:b, :], in_=src[:, a:b, :])
    return t


def _ptag(name):
    return "ps" if _OPT["psum_merged"] else name


def _expert_ffn(nc, wp, xp, hp, yp, tp, pp, x, g, u, d, y, C, first=False):
    """Emit one expert's FFN: y[H, C] = down( silu(g2)*g1*up ) for x[H, C]."""
    ct = _split_c(C)

    xr = x.rearrange("(kb p) c -> p kb c", p=128)  # [128, 8, C]
    gr = g.rearrange("(kb p) i -> p kb i", p=128)  # [128, 8, 4096]
    ur = u.rearrange("(kb p) i -> p kb i", p=128)  # [128, 8, 2048]
    dr = d.rearrange("(kb p) h -> p kb h", p=128)  # [128, 16, 1024]
    yr = y.rearrange("(hb p) c -> p hb c", p=128)  # [128, 8, C]

    hs_parts = _OPT["head_split"] if first else 1
    xs = xp.tile([128, KB_H, C], BF16, tag="xt", name="xs")
    sg2_0 = None
    if hs_parts > 1:
        # First expert: the PE's very first LDW/MM needs sg2[k=0] and
        # xs[k=0]. Interleave their k-slice DMAs so the earliest-needed
        # pieces land on distinct queues in the first round-robin wave.
        sg2_0 = wp.tile([128, KB_H, 1024], BF16, tag="w", name="sg2")
        for k in range(KB_H):
            nc.sync.dma_start(
                out=sg2_0[:, k : k + 1, :],
                in_=gr[:, k : k + 1, 2048 : 2048 + 1024],
            )
            nc.sync.dma_start(out=xs[:, k : k + 1, :], in_=xr[:, k : k + 1, :])
    else:
        nc.sync.dma_start(out=xs, in_=xr)

    hh = hp.tile([128, KB_I, C], BF16, tag="hh", name="hh")

    # ---- gate + up fused phase ----
    for half in range(2):  # hh i-tiles 0-7 / 8-15
        lo = half * 1024
        p = hs_parts if half == 0 else 1
        if half == 0 and sg2_0 is not None:
            sg2 = sg2_0
        else:
            sg2 = _load_slab(
                nc, wp, [128, KB_H, 1024],
                gr[:, :, 2048 + lo : 2048 + lo + 1024], "w", "sg2", parts=p,
            )
        sg1 = _load_slab(
            nc, wp, [128, KB_H, 1024], gr[:, :, lo : lo + 1024], "w", "sg1",
            parts=p,
        )
        su = _load_slab(
            nc, wp, [128, KB_H, 1024], ur[:, :, lo : lo + 1024], "w", "su",
            parts=p,
        )

        for il in range(8):
            i = half * 8 + il
            ms = slice(il * 128, (il + 1) * 128)
            for cc in _chunk2(ct):
                # --- g2 stream (silu half) ---
                pg2 = [
                    pp.tile([128, w], F32, tag=_ptag("pg2"), name="pg2") for (_, w) in cc
                ]
                for k in range(KB_H):
                    for j, (off, w) in enumerate(cc):
                        nc.tensor.matmul(
                            pg2[j],
                            sg2[:, k, ms],
                            xs[:, k, off : off + w],
                            start=(k == 0),
                            stop=(k == KB_H - 1),
                        )
                sil = []
                for j, (off, w) in enumerate(cc):
                    t = tp.tile([128, 512], BF16, tag="t", name="t")
                    nc.scalar.activation(
                        out=t[:, :w],
                        in_=pg2[j],
                        func=mybir.ActivationFunctionType.Silu,
                    )
                    sil.append(t)
                # --- g1 stream ---
                pg1 = [
                    pp.tile([128, w], F32, tag=_ptag("pg1"), name="pg1") for (_, w) in cc
                ]
                for k in range(KB_H):
                    for j, (off, w) in enumerate(cc):
                        nc.tensor.matmul(
                            pg1[j],
                            sg1[:, k, ms],
                            xs[:, k, off : off + w],
                            start=(k == 0),
                            stop=(k == KB_H - 1),
                        )
                g12 = []
                for j, (off, w) in enumerate(cc):
                    t2 = tp.tile([128, 512], BF16, tag="g12", name="t2")
                    nc.vector.tensor_mul(t2[:, :w], sil[j][:, :w], pg1[j])
                    g12.append(t2)
                # --- up stream ---
                pu = [
                    pp.tile([128, w], F32, tag=_ptag("pu"), name="pu") for (_, w) in cc
                ]
                for k in range(KB_H):
                    for j, (off, w) in enumerate(cc):
                        nc.tensor.matmul(
                            pu[j],
                            su[:, k, ms],
                            xs[:, k, off : off + w],
                            start=(k == 0),
                            stop=(k == KB_H - 1),
                        )
                for j, (off, w) in enumerate(cc):
                    nc.vector.tensor_mul(
                        hh[:, i, off : off + w], g12[j][:, :w], pu[j]
                    )

    # ---- down phase ----
    dlo = _load_slab(nc, wp, [128, KB_H, 1024], dr[:, 0:8, :], "w", "dlo")
    dhi = _load_slab(nc, wp, [128, KB_H, 1024], dr[:, 8:16, :], "w", "dhi")

    for h in range(8):
        ms = slice(h * 128, (h + 1) * 128)
        yl = yp.tile([128, C], F32, tag="y", name="yl")
        for cc in _chunk2(ct):
            pd = [
                pp.tile([128, w], F32, tag=_ptag("pd"), name="pd") for (_, w) in cc
            ]
            for k in range(KB_I):
                sl = dlo if k < 8 else dhi
                for j, (off, w) in enumerate(cc):
                    nc.tensor.matmul(
                        pd[j],
                        sl[:, k % 8, ms],
                        hh[:, k, off : off + w],
                        start=(k == 0),
                        stop=(k == KB_I - 1),
                    )
            for j, (off, w) in enumerate(cc):
                nc.vector.tensor_copy(yl[:, off : off + w], pd[j])
                if _OPT["y_chunk"]:
                    nc.sync.dma_start(
                        out=yr[:, h, off : off + w], in_=yl[:, off : off + w]
                    )
        if not _OPT["y_chunk"]:
            nc.sync.dma_start(out=yr[:, h, :], in_=yl)


def _build_nc(CA: int, CB: int, reps: int = 1):
    """Build + compile the 2-expert-slot SPMD program (same on all cores)."""
    nc = bacc.Bacc(
        "TRN2", target_bir_lowering=False, debug=False, num_devices=N_CORES
    )
    dram = {}
    for s, C in (("a", CA), ("b", CB)):
        dram[f"x{s}"] = nc.dram_tensor(
            f"x{s}", [H, C], BF16, kind="ExternalInput"
        ).ap()
        dram[f"g{s}"] = nc.dram_tensor(
            f"g{s}", [H, I_G], BF16, kind="ExternalInput"
        ).ap()
        dram[f"u{s}"] = nc.dram_tensor(
            f"u{s}", [H, I_H], BF16, kind="ExternalInput"
        ).ap()
        dram[f"d{s}"] = nc.dram_tensor(
            f"d{s}", [I_H, H], BF16, kind="ExternalInput"
        ).ap()
        dram[f"y{s}"] = nc.dram_tensor(
            f"y{s}", [H, C], F32, kind="ExternalOutput"
        ).ap()

    with tile.TileContext(nc) as tc:
        with (
            tc.tile_pool(name="wp", bufs=_OPT["wp_bufs"]) as wp,  # 16KB/part weight slabs
            tc.tile_pool(name="xp", bufs=2) as xp,
            tc.tile_pool(name="hp", bufs=1) as hp,
            tc.tile_pool(name="yp", bufs=_OPT["yp_bufs"]) as yp,
            tc.tile_pool(name="tp", bufs=_OPT["tp_bufs"]) as tp,
            tc.tile_pool(name="pp", bufs=(8 if _OPT["psum_merged"] else 2), space="PSUM") as pp,
        ):

            def body():
                for si, (s, C) in enumerate((("a", CA), ("b", CB))):
                    _expert_ffn(
                        nc,
                        wp,
                        xp,
                        hp,
                        yp,
                        tp,
                        pp,
                        dram[f"x{s}"],
                        dram[f"g{s}"],
                        dram[f"u{s}"],
                        dram[f"d{s}"],
                        dram[f"y{s}"],
                        C,
                        first=(si == 0),
                    )

            if reps == 1:
                body()
            else:
                ET = mybir.EngineType
                with tc.For_i(
                    0,
                    reps,
                    1,
                    hint_engines=(ET.PE, ET.DVE, ET.Activation, ET.SP, ET.Pool),
                ):
                    body()

    nc.compile()
    return nc


_NC_CACHE: dict = {}


def _get_nc(CA: int, CB: int):
    key = (CA, CB)
    if key not in _NC_CACHE:
        _NC_CACHE[key] = _build_nc(CA, CB)
    return _NC_CACHE[key]


def _route_and_shard(hs, rw, gw, uw, dw):
    """Host-side router + expert-parallel sharding of the full inputs."""
    B, S, _ = hs.shape
    T = B * S
    x = np.ascontiguousarray(hs.reshape(T, H)).astype(np.float32, copy=False)

    logits = x @ rw.astype(np.float32)  # [T, E]
    order = np.argsort(-logits, axis=1, kind="stable")[:, :2]
    l12 = np.take_along_axis(logits, order, axis=1).astype(np.float64)
    w1 = 1.0 / (1.0 + np.exp(l12[:, 1] - l12[:, 0]))  # renormalized top-2
    w2 = 1.0 - w1
    i1, i2 = order[:, 0], order[:, 1]

    idx, wts = [], []
    for e in range(E):
        m1 = i1 == e
        ide = np.nonzero(m1 | (i2 == e))[0]
        we = np.where(m1[ide], w1[ide], w2[ide]).astype(np.float32)
        idx.append(ide)
        wts.append(we)
    counts = np.array([len(v) for v in idx])

    # pair largest with smallest expert per core for load balance
    desc = np.argsort(-counts, kind="stable")
    slotA = [int(desc[c]) for c in range(N_CORES)]
    slotB = [int(desc[2 * N_CORES - 1 - c]) for c in range(N_CORES)]
    CA = max(256, _ceil_mult(int(counts[slotA].max()), 8))
    CB = max(256, _ceil_mult(int(counts[slotB].max()), 8))

    xT = np.ascontiguousarray(x.T).astype(NP_BF16)  # [H, T]

    in_maps = []
    for c in range(N_CORES):
        m = {}
        for s, e, C in (("a", slotA[c], CA), ("b", slotB[c], CB)):
            xe = np.zeros((H, C), NP_BF16)
            n = counts[e]
            xe[:, :n] = xT[:, idx[e]]
            m[f"x{s}"] = xe
            m[f"g{s}"] = np.ascontiguousarray(gw[e]).astype(NP_BF16)
            m[f"u{s}"] = np.ascontiguousarray(uw[e]).astype(NP_BF16)
            m[f"d{s}"] = np.ascontiguousarray(dw[e]).astype(NP_BF16)
        in_maps.append(m)

    meta = dict(
        B=B, S=S, T=T, idx=idx, wts=wts, counts=counts,
        slotA=slotA, slotB=slotB, CA=CA, CB=CB,
    )
    return in_maps, meta


def _combine(results, meta):
    """Host-side top-2 weighted combine (unshard)."""
    T = meta["T"]
    out = np.zeros((T, H), np.float32)
    for c in range(N_CORES):
        for s, e in (("a", meta["slotA"][c]), ("b", meta["slotB"][c])):
            n = int(meta["counts"][e])
            if n == 0:
                continue
            y = results[c][f"y{s}"][:, :n]  # [H, n] f32
            out[meta["idx"][e]] += meta["wts"][e][:, None] * y.T
    return out.reshape(meta["B"], meta["S"], H)


def _run_spmd(nc, in_maps):
    try:
        return bass_utils.run_bass_kernel_spmd(
            nc, in_maps, core_ids=list(range(N_CORES))
        )
    except ModuleNotFoundError:
        # axon NTFF profiling hook unavailable in this container; retry
        # with tracing force-disabled.
        os.environ["BASS_NEVER_TRACE"] = "1"
        try:
            return bass_utils.run_bass_kernel_spmd(
                nc, in_maps, core_ids=list(range(N_CORES))
            )
        finally:
            os.environ.pop("BASS_NEVER_TRACE", None)


def kernel(hidden_states, router_w, gate_w, up_w, down_w):
    hs = np.asarray(hidden_states)
    rw = np.asarray(router_w)
    gw = np.asarray(gate_w)
    uw = np.asarray(up_w)
    dw = np.asarray(down_w)

    in_maps, meta = _route_and_shard(hs, rw, gw, uw, dw)
    nc = _get_nc(meta["CA"], meta["CB"])
    res = _run_spmd(nc, in_maps)
    return _combine(res.results, meta)



# revision 5
# speedup vs baseline: 1.2796x; 1.1797x over previous
"""Expert-parallel MoE (top-2 of 16 experts) for Trainium2, 8 NeuronCores.

Sharding strategy (per spec sharding_hint): expert-parallel. The 16 experts'
gate/up/down weights are sharded 2-per-core across the 8 cores. The router
(a [T,16] matmul + top-2, ~0.01% of total FLOPs) runs on the host at input-
shard time; the "all-to-all token dispatch" is realized as the host-side
gather that builds each core's token batch, and the top-2 weighted combine
is the host-side scatter-add at unshard time.

On-device per core (all heavy FLOPs + memory traffic):
  for each of its 2 experts, with X_e^T [H, C] (tokens on the PE free dim):
    G1 = gate_w[:, :2048]^T-tiles @ X^T      (PSUM f32)
    G2 = silu(gate_w[:, 2048:] @ X^T)        (ScalarE silu from PSUM)
    HH = G2 * G1 * (up_w @ X^T)              (VectorE, bf16)
    Y^T = down_w-tiles @ HH                  (PSUM f32 -> SBUF -> HBM)

All matmuls in bf16 (weights cast at shard time), f32 accumulation.
Activations are laid out transposed ([H, C], tokens on the moving/free dim)
so every weight matrix is used in its natural [K, M] layout with zero
on-device transposes.
"""

import os

import numpy as np
import ml_dtypes

import concourse.tile as tile
import concourse.mybir as mybir
from concourse import bacc
from concourse import bass_utils

N_CORES = 8
E = 16
H = 1024
I_G = 4096  # gate projection width
I_H = 2048  # up/down inner width
KB_H = H // 128  # 8 k-tiles for H-contraction
KB_I = I_H // 128  # 16 k-tiles for I_H-contraction

# 16-bit matmul dtype: fp16 and bf16 run at the same PE rate (1 cyc/row);
# fp16's 10 mantissa bits give ~4x lower rounding error for this data
# (all values well inside fp16 range).
BF16 = mybir.dt.float16
F32 = mybir.dt.float32
NP_BF16 = np.float16


def _ceil_mult(n: int, m: int) -> int:
    return ((n + m - 1) // m) * m


def _split_c(C: int):
    """Split capacity C (multiple of 4) into PE free-dim tiles.

    Near-equal chunks, each <=512 (one PSUM bank in f32) and >=256 where
    possible (keeps the per-tile LDWEIGHTS (~107ns) hidden under the
    matmul stream). Returns list of (offset, width)."""
    assert C % 4 == 0 and C > 0
    n = (C + 511) // 512
    if _OPT.get("chunk_mode", "equal") == "wide":
        # maximal 512-wide chunks plus a (possibly small) remainder chunk
        widths = [512] * (C // 512)
        if C % 512:
            widths.append(C % 512)
    else:
        # near-equal chunks, widths mult-of-4
        widths = []
        acc = 0
        for i in range(n):
            w = C * (i + 1) // n - acc
            w = (w + 3) // 4 * 4
            w = min(w, C - acc)
            widths.append(w)
            acc += w
        assert acc == C
    assert sum(widths) == C and all(w <= 512 for w in widths)
    out = []
    off = 0
    for w in widths:
        out.append((off, w))
        off += w
    return out


def _chunk2(seq):
    return [seq[i : i + 2] for i in range(0, len(seq), 2)]


_OPT = dict(psum_merged=True, dma_split=False, y_chunk=False, head_split=8, wp_bufs=6, tp_bufs=4, yp_bufs=3)


def _load_slab(nc, pool, shape, src, tag, name, parts=2):
    """Allocate a [128, kb, n] slab and load it with `parts` DMAs split
    along the k dimension, so the first k-tiles land early and the PE can
    start before the whole slab arrives."""
    t = pool.tile(shape, BF16, tag=tag, name=name)
    kb = shape[1]
    step = max(1, kb // parts)
    for a in range(0, kb, step):
        b = min(kb, a + step)
        nc.sync.dma_start(out=t[:, a:b, :], in_=src[:, a:b, :])
    return t


def _ptag(name):
    return "ps" if _OPT["psum_merged"] else name


def _expert_ffn(nc, wp, xp, hp, yp, tp, pp, x, g, u, d, y, C, first=False):
    """Emit one expert's FFN: y[H, C] = down( silu(g2)*g1*up ) for x[H, C]."""
    ct = _split_c(C)

    xr = x.rearrange("(kb p) c -> p kb c", p=128)  # [128, 8, C]
    gr = g.rearrange("(kb p) i -> p kb i", p=128)  # [128, 8, 4096]
    ur = u.rearrange("(kb p) i -> p kb i", p=128)  # [128, 8, 2048]
    dr = d.rearrange("(kb p) h -> p kb h", p=128)  # [128, 16, 1024]
    yr = y.rearrange("(hb p) c -> p hb c", p=128)  # [128, 8, C]

    hs_parts = _OPT["head_split"] if first else 1
    xs = xp.tile([128, KB_H, C], BF16, tag="xt", name="xs")
    sg2_0 = None
    if hs_parts > 1:
        # First expert: the PE's very first LDW/MM needs sg2[k=0] and
        # xs[k=0]. Interleave their k-slice DMAs so the earliest-needed
        # pieces land on distinct queues in the first round-robin wave.
        sg2_0 = wp.tile([128, KB_H, 1024], BF16, tag="w", name="sg2")
        for k in range(KB_H):
            nc.sync.dma_start(
                out=sg2_0[:, k : k + 1, :],
                in_=gr[:, k : k + 1, 2048 : 2048 + 1024],
            )
            nc.sync.dma_start(out=xs[:, k : k + 1, :], in_=xr[:, k : k + 1, :])
    else:
        nc.sync.dma_start(out=xs, in_=xr)

    hh = hp.tile([128, KB_I, C], BF16, tag="hh", name="hh")

    # ---- gate + up fused phase ----
    for half in range(2):  # hh i-tiles 0-7 / 8-15
        lo = half * 1024
        p = hs_parts if half == 0 else 1
        if half == 0 and sg2_0 is not None:
            sg2 = sg2_0
        else:
            sg2 = _load_slab(
                nc, wp, [128, KB_H, 1024],
                gr[:, :, 2048 + lo : 2048 + lo + 1024], "w", "sg2", parts=p,
            )
        sg1 = _load_slab(
            nc, wp, [128, KB_H, 1024], gr[:, :, lo : lo + 1024], "w", "sg1",
            parts=p,
        )
        su = _load_slab(
            nc, wp, [128, KB_H, 1024], ur[:, :, lo : lo + 1024], "w", "su",
            parts=p,
        )

        for il in range(8):
            i = half * 8 + il
            ms = slice(il * 128, (il + 1) * 128)
            for cc in _chunk2(ct):
                # --- g2 stream (silu half) ---
                pg2 = [
                    pp.tile([128, w], F32, tag=_ptag("pg2"), name="pg2") for (_, w) in cc
                ]
                for k in range(KB_H):
                    for j, (off, w) in enumerate(cc):
                        nc.tensor.matmul(
                            pg2[j],
                            sg2[:, k, ms],
                            xs[:, k, off : off + w],
                            start=(k == 0),
                            stop=(k == KB_H - 1),
                        )
                sil = []
                for j, (off, w) in enumerate(cc):
                    t = tp.tile([128, 512], BF16, tag="t", name="t")
                    nc.scalar.activation(
                        out=t[:, :w],
                        in_=pg2[j],
                        func=mybir.ActivationFunctionType.Silu,
                    )
                    sil.append(t)
                # --- g1 stream ---
                pg1 = [
                    pp.tile([128, w], F32, tag=_ptag("pg1"), name="pg1") for (_, w) in cc
                ]
                for k in range(KB_H):
                    for j, (off, w) in enumerate(cc):
                        nc.tensor.matmul(
                            pg1[j],
                            sg1[:, k, ms],
                            xs[:, k, off : off + w],
                            start=(k == 0),
                            stop=(k == KB_H - 1),
                        )
                g12 = []
                for j, (off, w) in enumerate(cc):
                    t2 = tp.tile([128, 512], BF16, tag="g12", name="t2")
                    nc.vector.tensor_mul(t2[:, :w], sil[j][:, :w], pg1[j])
                    g12.append(t2)
                # --- up stream ---
                pu = [
                    pp.tile([128, w], F32, tag=_ptag("pu"), name="pu") for (_, w) in cc
                ]
                for k in range(KB_H):
                    for j, (off, w) in enumerate(cc):
                        nc.tensor.matmul(
                            pu[j],
                            su[:, k, ms],
                            xs[:, k, off : off + w],
                            start=(k == 0),
                            stop=(k == KB_H - 1),
                        )
                for j, (off, w) in enumerate(cc):
                    nc.vector.tensor_mul(
                        hh[:, i, off : off + w], g12[j][:, :w], pu[j]
                    )

    # ---- down phase ----
    dlo = _load_slab(nc, wp, [128, KB_H, 1024], dr[:, 0:8, :], "w", "dlo")
    dhi = _load_slab(nc, wp, [128, KB_H, 1024], dr[:, 8:16, :], "w", "dhi")

    for h in range(8):
        ms = slice(h * 128, (h + 1) * 128)
        yl = yp.tile([128, C], F32, tag="y", name="yl")
        for cc in _chunk2(ct):
            pd = [
                pp.tile([128, w], F32, tag=_ptag("pd"), name="pd") for (_, w) in cc
            ]
            for k in range(KB_I):
                sl = dlo if k < 8 else dhi
                for j, (off, w) in enumerate(cc):
                    nc.tensor.matmul(
                        pd[j],
                        sl[:, k % 8, ms],
                        hh[:, k, off : off + w],
                        start=(k == 0),
                        stop=(k == KB_I - 1),
                    )
            for j, (off, w) in enumerate(cc):
                nc.vector.tensor_copy(yl[:, off : off + w], pd[j])
                if _OPT["y_chunk"]:
                    nc.sync.dma_start(
                        out=yr[:, h, off : off + w], in_=yl[:, off : off + w]
                    )
        if not _OPT["y_chunk"]:
            nc.sync.dma_start(out=yr[:, h, :], in_=yl)


def _build_nc(CA: int, CB: int, reps: int = 1):
    """Build + compile the 2-expert-slot SPMD program (same on all cores)."""
    nc = bacc.Bacc(
        "TRN2", target_bir_lowering=False, debug=False, num_devices=N_CORES
    )
    dram = {}
    for s, C in (("a", CA), ("b", CB)):
        dram[f"x{s}"] = nc.dram_tensor(
            f"x{s}", [H, C], BF16, kind="ExternalInput"
        ).ap()
        dram[f"g{s}"] = nc.dram_tensor(
            f"g{s}", [H, I_G], BF16, kind="ExternalInput"
        ).ap()
        dram[f"u{s}"] = nc.dram_tensor(
            f"u{s}", [H, I_H], BF16, kind="ExternalInput"
        ).ap()
        dram[f"d{s}"] = nc.dram_tensor(
            f"d{s}", [I_H, H], BF16, kind="ExternalInput"
        ).ap()
        dram[f"y{s}"] = nc.dram_tensor(
            f"y{s}", [H, C], F32, kind="ExternalOutput"
        ).ap()

    with tile.TileContext(nc) as tc:
        with (
            tc.tile_pool(name="wp", bufs=_OPT["wp_bufs"]) as wp,  # 16KB/part weight slabs
            tc.tile_pool(name="xp", bufs=2) as xp,
            tc.tile_pool(name="hp", bufs=1) as hp,
            tc.tile_pool(name="yp", bufs=_OPT["yp_bufs"]) as yp,
            tc.tile_pool(name="tp", bufs=_OPT["tp_bufs"]) as tp,
            tc.tile_pool(name="pp", bufs=(8 if _OPT["psum_merged"] else 2), space="PSUM") as pp,
        ):

            def body():
                for si, (s, C) in enumerate((("a", CA), ("b", CB))):
                    _expert_ffn(
                        nc,
                        wp,
                        xp,
                        hp,
                        yp,
                        tp,
                        pp,
                        dram[f"x{s}"],
                        dram[f"g{s}"],
                        dram[f"u{s}"],
                        dram[f"d{s}"],
                        dram[f"y{s}"],
                        C,
                        first=(si == 0),
                    )

            if reps == 1:
                body()
            else:
                ET = mybir.EngineType
                with tc.For_i(
                    0,
                    reps,
                    1,
                    hint_engines=(ET.PE, ET.DVE, ET.Activation, ET.SP, ET.Pool),
                ):
                    body()

    nc.compile()
    return nc


_NC_CACHE: dict = {}


def _get_nc(CA: int, CB: int):
    key = (CA, CB)
    if key not in _NC_CACHE:
        _NC_CACHE[key] = _build_nc(CA, CB)
    return _NC_CACHE[key]


def _route_and_shard(hs, rw, gw, uw, dw):
    """Host-side router + expert-parallel sharding of the full inputs."""
    B, S, _ = hs.shape
    T = B * S
    x = np.ascontiguousarray(hs.reshape(T, H)).astype(np.float32, copy=False)

    logits = x @ rw.astype(np.float32)  # [T, E]
    order = np.argsort(-logits, axis=1, kind="stable")[:, :2]
    l12 = np.take_along_axis(logits, order, axis=1).astype(np.float64)
    w1 = 1.0 / (1.0 + np.exp(l12[:, 1] - l12[:, 0]))  # renormalized top-2
    w2 = 1.0 - w1
    i1, i2 = order[:, 0], order[:, 1]

    idx, wts = [], []
    for e in range(E):
        m1 = i1 == e
        ide = np.nonzero(m1 | (i2 == e))[0]
        we = np.where(m1[ide], w1[ide], w2[ide]).astype(np.float32)
        idx.append(ide)
        wts.append(we)
    counts = np.array([len(v) for v in idx])

    # pair largest with smallest expert per core for load balance
    desc = np.argsort(-counts, kind="stable")
    slotA = [int(desc[c]) for c in range(N_CORES)]
    slotB = [int(desc[2 * N_CORES - 1 - c]) for c in range(N_CORES)]
    CA = max(256, _ceil_mult(int(counts[slotA].max()), 4))
    CB = max(256, _ceil_mult(int(counts[slotB].max()), 4))

    xT = np.ascontiguousarray(x.T).astype(NP_BF16)  # [H, T]

    in_maps = []
    for c in range(N_CORES):
        m = {}
        for s, e, C in (("a", slotA[c], CA), ("b", slotB[c], CB)):
            xe = np.zeros((H, C), NP_BF16)
            n = counts[e]
            xe[:, :n] = xT[:, idx[e]]
            m[f"x{s}"] = xe
            m[f"g{s}"] = np.ascontiguousarray(gw[e]).astype(NP_BF16)
            m[f"u{s}"] = np.ascontiguousarray(uw[e]).astype(NP_BF16)
            m[f"d{s}"] = np.ascontiguousarray(dw[e]).astype(NP_BF16)
        in_maps.append(m)

    meta = dict(
        B=B, S=S, T=T, idx=idx, wts=wts, counts=counts,
        slotA=slotA, slotB=slotB, CA=CA, CB=CB,
    )
    return in_maps, meta


def _combine(results, meta):
    """Host-side top-2 weighted combine (unshard)."""
    T = meta["T"]
    out = np.zeros((T, H), np.float32)
    for c in range(N_CORES):
        for s, e in (("a", meta["slotA"][c]), ("b", meta["slotB"][c])):
            n = int(meta["counts"][e])
            if n == 0:
                continue
            y = results[c][f"y{s}"][:, :n]  # [H, n] f32
            out[meta["idx"][e]] += meta["wts"][e][:, None] * y.T
    return out.reshape(meta["B"], meta["S"], H)


def _run_spmd(nc, in_maps):
    try:
        return bass_utils.run_bass_kernel_spmd(
            nc, in_maps, core_ids=list(range(N_CORES))
        )
    except ModuleNotFoundError:
        # axon NTFF profiling hook unavailable in this container; retry
        # with tracing force-disabled.
        os.environ["BASS_NEVER_TRACE"] = "1"
        try:
            return bass_utils.run_bass_kernel_spmd(
                nc, in_maps, core_ids=list(range(N_CORES))
            )
        finally:
            os.environ.pop("BASS_NEVER_TRACE", None)


def kernel(hidden_states, router_w, gate_w, up_w, down_w):
    hs = np.asarray(hidden_states)
    rw = np.asarray(router_w)
    gw = np.asarray(gate_w)
    uw = np.asarray(up_w)
    dw = np.asarray(down_w)

    in_maps, meta = _route_and_shard(hs, rw, gw, uw, dw)
    nc = _get_nc(meta["CA"], meta["CB"])
    res = _run_spmd(nc, in_maps)
    return _combine(res.results, meta)

